# revision 1
# baseline (speedup 1.0000x reference)
"""DFlashAttention Trainium2 kernel (8 NeuronCores).

Sharding: batch (2) data-parallel x kv-head-group (4) tensor-parallel = 8 cores.
Core c handles batch b=c//4, kv head g=c%4, q heads [4g..4g+4).
Host pre-transposes all operands so every on-device matmul contraction dim is
already the partition dim; o_proj partials are summed on host (the all-reduce).

Device pipeline per core (software-pipelined over 9 kv blocks of 512):
  QT = WqT.T @ XdT            -> [hd*4, 512] per-head RMSNorm + RoPE (PE-bcast trick)
  per 512-wide kv block:
    KT/VT = W{k,v}T.T @ XkvT  -> [128, 512];  K: RMSNorm + RoPE;  V: PE-transpose
    ST[c,q] = KT_tile.T @ QT  -> exp on ACT (no max subtraction; scores bounded ~5.3)
    pacc[h] += P              (softmax denominators accumulated on GPSIMD)
    OT[hd,q] += V_tile.T @ P  (flash accumulation in PSUM, unnormalized)
  denom = ones.T @ pacc;  OT /= denom (PE broadcast of reciprocal)
  out = OT.T @ WoT (partial; host sums the 4 cores of each batch = all-reduce)

dtypes: bf16 operands for all PE matmuls except the RoPE rotate-half permutation,
V-transposes and o_proj-normalize helpers (fp32r; bf16 corrupts those on HW),
fp32 PSUM accumulation throughout, fp32 output.
"""

import numpy as np
import ml_dtypes

import concourse.bass as bass
import concourse.mybir as mybir
from concourse import bacc
from concourse.tile import TileContext
from concourse import bass_utils

F32 = mybir.dt.float32
F32R = mybir.dt.float32r
BF16 = mybir.dt.bfloat16

B, CTX, DRAFT, D = 2, 4096, 512, 2048
H, KVH, HD = 16, 4, 128
NH = H // KVH            # 4 q heads per core
TOT = CTX + DRAFT        # 4608
BLK = 512
NB = TOT // BLK          # 9 kv blocks
SQ = DRAFT               # 512 queries
EPS = 1e-6
THETA = 10000.0
SCALE = 1.0 / float(np.sqrt(HD))

_CACHE: dict = {}


def _build_nc(repeat: int = 1, pend_depth: int = 4, pex_bufs: int = 8, qk_bf16: bool = True,
              st_bufs: int = 3, ot_bufs: int = 1, pool_elem: bool = True, x4_bufs: int = 8):
    nc = bacc.Bacc()

    xd = nc.dram_tensor("xd", [D, SQ], BF16, kind="ExternalInput")
    xkv = nc.dram_tensor("xkv", [D, TOT], BF16, kind="ExternalInput")
    wq = nc.dram_tensor("wq", [D, NH * HD], BF16, kind="ExternalInput")
    wk = nc.dram_tensor("wk", [D, HD], BF16, kind="ExternalInput")
    wv = nc.dram_tensor("wv", [D, HD], BF16, kind="ExternalInput")
    wo = nc.dram_tensor("wo", [NH * HD, D], BF16, kind="ExternalInput")
    cosk_d = nc.dram_tensor("cosk", [HD, TOT], BF16, kind="ExternalInput")
    sink_d = nc.dram_tensor("sink", [HD, TOT], BF16, kind="ExternalInput")
    perm_d = nc.dram_tensor("perm", [HD, HD], F32R, kind="ExternalInput")
    ident_d = nc.dram_tensor("ident", [HD, HD], F32R, kind="ExternalInput")
    onesc_d = nc.dram_tensor("onesc", [HD, 1], F32R, kind="ExternalInput")
    onesr_d = nc.dram_tensor("onesr", [1, HD], F32R, kind="ExternalInput")
    wqn_d = nc.dram_tensor("wqn", [1, HD], F32R, kind="ExternalInput")
    wkn_d = nc.dram_tensor("wkn", [1, HD], F32R, kind="ExternalInput")
    out = nc.dram_tensor("out", [SQ, D], F32, kind="ExternalOutput")

    with nc.allow_low_precision("f32r rounding required by fp32r matmul consumers"), \
         TileContext(nc) as tc:
        with (
            tc.tile_pool(name="const", bufs=1) as cpool,
            tc.tile_pool(name="big", bufs=1) as bpool,
            tc.tile_pool(name="x4", bufs=x4_bufs) as x4pool,       # [128,4,512] streams
            tc.tile_pool(name="w4", bufs=4) as w4pool,       # woN (phase3, prefetched)
            tc.tile_pool(name="scr", bufs=2) as scr,         # norm/rope scratch
            tc.tile_pool(name="pex", bufs=pex_bufs) as pex,         # exp probs
            tc.tile_pool(name="acc", bufs=1) as accp,        # persistent sbuf accumulators
            tc.tile_pool(name="ps_proj", bufs=2, space="PSUM") as ps_proj,
            tc.tile_pool(name="ps_nrm", bufs=2, space="PSUM") as ps_nrm,
            tc.tile_pool(name="ps_st", bufs=st_bufs, space="PSUM") as ps_st,
            tc.tile_pool(name="ps_ot", bufs=ot_bufs, space="PSUM") as ps_ot,
        ):
            SDT = BF16 if qk_bf16 else F32R
            # persistent accumulators
            otsb = [accp.tile([HD, SQ], F32, name=f"otsb{h}") for h in range(NH)]
            pacc = [accp.tile([128, SQ], F32, name=f"pacc{h}") for h in range(NH)]
            qrope = [accp.tile([HD, SQ], SDT, name=f"qrope{h}") for h in range(NH)]

            def norm_rope(src_ps, wrow, cos_sb, sin_sb, csl, dst, tagpfx):
                """RMSNorm (per position, over partition dim) + RoPE on a
                [128, 512] tile in PSUM; writes f32r SBUF tile `dst`."""
                src_sb = scr.tile([128, BLK], F32, name=f"{tagpfx}_src", tag="srcsb")
                nc.vector.tensor_copy(src_sb[:, :], src_ps[:, :])
                sq = scr.tile([128, BLK], F32R, name=f"{tagpfx}_sq", tag="sq")
                if pool_elem:
                    nc.gpsimd.tensor_mul(sq[:, :], src_sb[:, :], src_sb[:, :])
                else:
                    nc.scalar.square(sq[:, :], src_sb[:, :])
                ssq = ps_nrm.tile([1, BLK], F32, name=f"{tagpfx}_ssq", tag="nrm")
                nc.tensor.matmul(ssq[:, :], onesc[:, :], sq[:, :], start=True, stop=True)
                srt = scr.tile([1, BLK], F32, name=f"{tagpfx}_srt", tag="rs")
                nc.scalar.activation(srt[:, :], ssq[:, :],
                                     mybir.ActivationFunctionType.Sqrt,
                                     bias=eps_t[:, :], scale=1.0 / HD)
                rs = scr.tile([1, BLK], F32R, name=f"{tagpfx}_rs", tag="rs")
                nc.vector.reciprocal(rs[:, :], srt[:, :])
                nf = ps_nrm.tile([128, BLK], F32, name=f"{tagpfx}_nf", tag="nrm")
                nc.tensor.matmul(nf[:, :], wrow[:, :], rs[:, :], start=True, stop=True)
                xn = scr.tile([128, BLK], F32R, name=f"{tagpfx}_xn", tag="xn")
                nc.vector.tensor_mul(xn[:, :], src_sb[:, :], nf[:, :])
                # rope: dst = xn*cos + (perm @ xn)*sin
                pr = ps_nrm.tile([128, BLK], F32, name=f"{tagpfx}_pr", tag="nrm")
                nc.tensor.matmul(pr[:, :], perm[:, :], xn[:, :], start=True, stop=True)
                t1 = scr.tile([128, BLK], F32, name=f"{tagpfx}_t1", tag="t1")
                if pool_elem:
                    nc.gpsimd.tensor_mul(t1[:, :], xn[:, :], cos_sb[:, csl])
                else:
                    nc.vector.tensor_mul(t1[:, :], xn[:, :], cos_sb[:, csl])
                t2 = scr.tile([128, BLK], F32, name=f"{tagpfx}_t2", tag="sq")
                nc.vector.tensor_mul(t2[:, :], pr[:, :], sin_sb[:, csl])
                nc.vector.tensor_add(dst[:, :], t1[:, :], t2[:, :])

            # ---- phase 1: Q projection DMAs + MMs ----
            xd4 = []
            wq4 = []
            for dg in range(4):
                xt = x4pool.tile([128, 4, BLK], BF16, name=f"xd4_{dg}", tag="x4")
                nc.sync.dma_start(
                    xt[:, :, :],
                    xd[dg * 512:(dg + 1) * 512, :].rearrange("(j p) c -> p j c", p=128))
                xd4.append(xt)
                wt = x4pool.tile([128, 4, BLK], BF16, name=f"wq4_{dg}", tag="x4")
                nc.sync.dma_start(
                    wt[:, :, :],
                    wq[dg * 512:(dg + 1) * 512, :].rearrange("(j p) c -> p j c", p=128))
                wq4.append(wt)
            # ---- constants / tables ----
            perm = cpool.tile([HD, HD], F32R, name="perm_sb")
            nc.sync.dma_start(perm[:, :], perm_d[:, :])
            ident = cpool.tile([HD, HD], F32R, name="ident_sb")
            nc.sync.dma_start(ident[:, :], ident_d[:, :])
            onesc = cpool.tile([HD, 1], F32R, name="onesc_sb")
            nc.sync.dma_start(onesc[:, :], onesc_d[:, :])
            onesr = cpool.tile([1, HD], F32R, name="onesr_sb")
            nc.sync.dma_start(onesr[:, :], onesr_d[:, :])
            wqn = cpool.tile([1, HD], F32R, name="wqn_sb")
            nc.sync.dma_start(wqn[:, :], wqn_d[:, :])
            wkn = cpool.tile([1, HD], F32R, name="wkn_sb")
            nc.sync.dma_start(wkn[:, :], wkn_d[:, :])
            eps_t = cpool.tile([1, 1], F32, name="eps_sb")
            nc.vector.memset(eps_t[:, :], EPS)
            onescb = cpool.tile([HD, 1], BF16, name="onescb_sb")
            nc.vector.memset(onescb[:, :], 1.0)
            wk_sb = bpool.tile([128, 16, HD], BF16, name="wk_sb")
            nc.sync.dma_start(wk_sb[:, :, :], wk[:, :].rearrange("(g p) h -> p g h", p=128))
            wv_sb = bpool.tile([128, 16, HD], BF16, name="wv_sb")
            nc.sync.dma_start(wv_sb[:, :, :], wv[:, :].rearrange("(g p) h -> p g h", p=128))

            SDT = BF16 if qk_bf16 else F32R
            psqs = []
            for h in range(NH):
                psq = ps_st.tile([HD, SQ], F32, name=f"psq{h}", tag="st")
                for dg in range(4):
                    for j in range(4):
                        nc.tensor.matmul(
                            psq[:, :],
                            wq4[dg][:, j, h * HD:(h + 1) * HD],
                            xd4[dg][:, j, :],
                            start=(dg == 0 and j == 0),
                            stop=(dg == 3 and j == 3))
                psqs.append(psq)

            # ---- phase 2: software-pipelined kv blocks ----
            # PE emission order per block b:
            #   [K/V proj MMs b] ... [norm-chain MMs b-1 + V transposes b-1]
            #   ... [scores/exp/denom/attn b-2, 2-ahead st emission]
            # so ACT/DVE chain latencies hide behind dense projection MMs.
            state: dict = {}
            consts: dict = {}
            pfx = [""]

            def load_block(cb):
                csl = slice(cb * BLK, (cb + 1) * BLK)
                xk4 = []
                for dg in range(4):
                    xt = x4pool.tile([128, 4, BLK], BF16, name=f"{pfx[0]}xk4_{cb}_{dg}", tag="x4")
                    nc.sync.dma_start(
                        xt[:, :, :],
                        xkv[dg * 512:(dg + 1) * 512, csl].rearrange("(j p) c -> p j c", p=128))
                    xk4.append(xt)
                state[("xk4", cb)] = xk4

            def proj_block(cb):
                xk4 = state.pop(("xk4", cb))
                kt_ps = ps_proj.tile([HD, BLK], F32, name=f"{pfx[0]}kt{cb}", tag="proj")
                for dg in range(4):
                    for j in range(4):
                        nc.tensor.matmul(kt_ps[:, :], wk_sb[:, dg * 4 + j, :],
                                         xk4[dg][:, j, :],
                                         start=(dg == 0 and j == 0),
                                         stop=(dg == 3 and j == 3))
                vt_ps = ps_proj.tile([HD, BLK], F32, name=f"{pfx[0]}vt{cb}", tag="proj")
                for dg in range(4):
                    for j in range(4):
                        nc.tensor.matmul(vt_ps[:, :], wv_sb[:, dg * 4 + j, :],
                                         xk4[dg][:, j, :],
                                         start=(dg == 0 and j == 0),
                                         stop=(dg == 3 and j == 3))
                state[("kt", cb)] = kt_ps
                state[("vt", cb)] = vt_ps

            def prep_block(cb):
                """norm+rope on K, transpose V — chain MMs for block cb."""
                cosk = consts["cosk"]
                sink = consts["sink"]
                csl = slice(cb * BLK, (cb + 1) * BLK)
                kt_ps = state.pop(("kt", cb))
                vt_ps = state.pop(("vt", cb))
                ktf = scr.tile([HD, BLK], SDT, name=f"{pfx[0]}ktf{cb}", tag="ktf", bufs=2)
                norm_rope(kt_ps, wkn, cosk, sink, csl, ktf, f"{pfx[0]}k{cb}")
                vt_sb = scr.tile([HD, BLK], F32R, name=f"{pfx[0]}vt_sb{cb}", tag="vtsb")
                nc.vector.tensor_copy(vt_sb[:, :], vt_ps[:, :])
                tr_ps = ps_proj.tile([128, BLK], F32R, name=f"{pfx[0]}tr{cb}", tag="proj")
                vnat = []
                for j in range(4):
                    nc.tensor.transpose(tr_ps[:, j * HD:(j + 1) * HD],
                                        vt_sb[:, j * HD:(j + 1) * HD], ident[:, :])
                for j in range(4):
                    vn = scr.tile([128, HD], SDT, name=f"{pfx[0]}vn{cb}_{j}", tag=f"vn{j}", bufs=1)
                    nc.vector.tensor_copy(vn[:, :], tr_ps[:, j * HD:(j + 1) * HD])
                    vnat.append(vn)
                state[("ktf", cb)] = ktf
                state[("vnat", cb)] = vnat

            otn = [None] * NH

            def normalize_head(h):
                dsq = scr.tile([128, SQ], F32R, name=f"dsq{h}", tag="sq")
                nc.gpsimd.tensor_copy(dsq[:, :], pacc[h][:, :])
                den_ps = ps_nrm.tile([1, SQ], F32, name=f"den{h}", tag="nrm")
                lastp = state.pop(("lastpe", h), [])
                nc.tensor.matmul(den_ps[:, :], onesc[:, :], dsq[:, :],
                                 start=True, stop=(len(lastp) == 0))
                for i, pe_t in enumerate(lastp):
                    nc.tensor.matmul(den_ps[:, :], onescb[:, :], pe_t[:, :],
                                     start=False, stop=(i == len(lastp) - 1))
                rdt = scr.tile([1, SQ], F32, name=f"rdt{h}", tag="rs")
                nc.vector.tensor_copy(rdt[:, :], den_ps[:, :])
                rd = scr.tile([1, SQ], F32R, name=f"rd{h}", tag="rs")
                nc.vector.reciprocal(rd[:, :], rdt[:, :])
                nf = ps_nrm.tile([128, SQ], F32, name=f"onf{h}", tag="nrm")
                nc.tensor.matmul(nf[:, :], onesr[:, :], rd[:, :], start=True, stop=True)
                ot = accp.tile([HD, SQ], BF16, name=f"otn{h}")
                nc.vector.tensor_mul(ot[:, :], otsb[h][:, :], nf[:, :])
                otn[h] = ot

            def attn_block(cb):
                ktf = state.pop(("ktf", cb))
                vnat = state.pop(("vnat", cb))
                # 2-ahead pipeline: st MMs run ahead of exp-dependent den/ot MMs
                pend = []

                def flush_one():
                    h, j, p_exp, ot_ps = pend.pop(0)
                    nc.tensor.matmul(ot_ps[:, :], vnat[j][:, :], p_exp[:, :],
                                     start=(j == 0), stop=(j == 3))
                    last = cb == NB - 1 and state.get("last_rep")
                    if last:
                        # last block: denominator goes through PE in normalize_head
                        state.setdefault(("lastpe", h), []).append(p_exp)
                    elif cb == 0 and j == 0:
                        nc.gpsimd.tensor_copy(pacc[h][:, :], p_exp[:, :])
                    else:
                        nc.gpsimd.tensor_add(pacc[h][:, :], pacc[h][:, :], p_exp[:, :])
                    if j == 3:
                        if cb == 0:
                            nc.vector.tensor_copy(otsb[h][:, :], ot_ps[:, :])
                        else:
                            nc.vector.tensor_add(otsb[h][:, :], otsb[h][:, :], ot_ps[:, :])
                        if last:
                            normalize_head(h)

                for h in range(NH):
                    ot_ps = ps_ot.tile([HD, SQ], F32, name=f"{pfx[0]}ot{cb}_{h}", tag="ot")
                    for j in range(4):
                        st_ps = ps_st.tile([128, SQ], F32, name=f"{pfx[0]}st{cb}_{h}_{j}", tag="st")
                        nc.tensor.matmul(st_ps[:, :], ktf[:, j * HD:(j + 1) * HD],
                                         qrope[h][:, :], start=True, stop=True)
                        p_exp = pex.tile([128, SQ], SDT, name=f"{pfx[0]}pe{cb}_{h}_{j}", tag="pex")
                        nc.scalar.activation(p_exp[:, :], st_ps[:, :],
                                             mybir.ActivationFunctionType.Exp,
                                             scale=SCALE)
                        pend.append((h, j, p_exp, ot_ps))
                        if len(pend) >= pend_depth:
                            flush_one()
                while pend:
                    flush_one()

            # pipeline schedule
            for rep in range(repeat):
                pfx[0] = f"r{rep}_" if repeat > 1 else ""
                state["last_rep"] = (rep == repeat - 1)
                if rep == 0:
                    cosk = bpool.tile([HD, TOT], BF16, name="cosk_sb")
                    nc.sync.dma_start(cosk[:, :], cosk_d[:, :])
                    sink = bpool.tile([HD, TOT], BF16, name="sink_sb")
                    nc.sync.dma_start(sink[:, :], sink_d[:, :])
                    consts["cosk"] = cosk
                    consts["sink"] = sink
                load_block(0)
                load_block(1)
                if rep == 0:
                    pass
                cosk = consts["cosk"]
                sink = consts["sink"]
                proj_block(0)
                if rep == 0:
                    # Q norm chains (ACT/DVE work started during projections)
                    for h in range(NH):
                        norm_rope(psqs[h], wqn, cosk, sink, slice(CTX, TOT), qrope[h], f"q{h}")
                proj_block(1)
                prep_block(0)
                for cb in range(NB):
                    if cb + 2 < NB:
                        load_block(cb + 2)
                    if rep == repeat - 1 and cb == NB - 2:
                        for n in range(4):
                            woN = w4pool.tile([128, 4, 512], BF16, name=f"woN{n}", tag="w4")
                            nc.sync.dma_start(
                                woN[:, :, :],
                                wo[:, n * 512:(n + 1) * 512].rearrange("(h p) c -> p h c", p=128))
                            consts[f"woN{n}"] = woN
                    attn_block(cb)
                    if cb + 1 < NB:
                        prep_block(cb + 1)
                    if cb + 2 < NB:
                        proj_block(cb + 2)

            # ---- phase 3: o_proj (otn produced inside the last attn block) ----
            osbm = [scr.tile([128, D], F32, name=f"osbm{m}", tag=f"osbm{m}", bufs=1)
                    for m in range(4)]
            for n in range(4):
                nsl = slice(n * 512, (n + 1) * 512)
                woN = consts[f"woN{n}"]
                for m in range(4):
                    po = ps_st.tile([128, 512], F32, name=f"po{n}_{m}", tag="st")
                    for h in range(NH):
                        nc.tensor.matmul(po[:, :],
                                         otn[h][:, m * HD:(m + 1) * HD],
                                         woN[:, h, :],
                                         start=(h == 0), stop=(h == 3))
                    nc.vector.tensor_copy(osbm[m][:, nsl], po[:, :])
            for m in range(4):
                nc.sync.dma_start(out[m * 128:(m + 1) * 128, :], osbm[m][:, :])
    nc.finalize()
    return nc


def get_nc(repeat: int = 1, **kw):
    key = ("nc", repeat, tuple(sorted(kw.items())))
    if key not in _CACHE:
        _CACHE[key] = _build_nc(repeat, **kw)
    return _CACHE[key]


def _host_tables():
    inv = 1.0 / (THETA ** (np.arange(0, HD, 2, dtype=np.float32) / np.float32(HD)))
    inv2 = np.concatenate([inv, inv]).astype(np.float32)  # [128]
    pm = np.zeros((HD, HD), np.float32)
    pm[np.arange(64) + 64, np.arange(64)] = -1.0
    pm[np.arange(64), np.arange(64) + 64] = 1.0
    ident = np.eye(HD, dtype=np.float32)
    onesc = np.ones((HD, 1), np.float32)
    onesr = np.ones((1, HD), np.float32)
    return inv2, pm, ident, onesc, onesr


def _make_in_maps(inputs):
    draft = np.ascontiguousarray(np.asarray(inputs["draft_hidden"], np.float32))
    ctx = np.ascontiguousarray(np.asarray(inputs["context_hidden"], np.float32))
    Wq = np.asarray(inputs["Wq"], np.float32)
    Wk = np.asarray(inputs["Wk"], np.float32)
    Wv = np.asarray(inputs["Wv"], np.float32)
    Wo = np.asarray(inputs["Wo"], np.float32)
    qnw = np.asarray(inputs["q_norm_w"], np.float32).reshape(1, HD)
    knw = np.asarray(inputs["k_norm_w"], np.float32).reshape(1, HD)
    cpos = np.asarray(inputs["context_position_ids"])
    dpos = np.asarray(inputs["draft_position_ids"])

    inv2, pm, ident, onesc, onesr = _host_tables()

    in_maps = []
    for c in range(8):
        b, g = c // 4, c % 4
        kvin = np.concatenate([ctx[b], draft[b]], axis=0)       # [4608, 2048]
        xkvT = np.ascontiguousarray(kvin.T)                      # [2048, 4608]
        xdT = np.ascontiguousarray(draft[b].T)                   # [2048, 512]
        wqT = np.ascontiguousarray(Wq[4 * g * HD:(4 * g + 4) * HD, :].T)  # [2048, 512]
        wkT = np.ascontiguousarray(Wk[g * HD:(g + 1) * HD, :].T)          # [2048, 128]
        wvT = np.ascontiguousarray(Wv[g * HD:(g + 1) * HD, :].T)
        woT = np.ascontiguousarray(Wo[:, 4 * g * HD:(4 * g + 4) * HD].T)  # [512, 2048]
        fpos = np.concatenate([cpos[b], dpos[b]]).astype(np.float32)      # [4608]
        angk = inv2[:, None] * fpos[None, :]
        bf = ml_dtypes.bfloat16
        in_maps.append({
            "xd": xdT.astype(bf), "xkv": xkvT.astype(bf), "wq": wqT.astype(bf),
            "wk": wkT.astype(bf), "wv": wvT.astype(bf), "wo": woT.astype(bf),
            "cosk": np.cos(angk).astype(bf),
            "sink": np.sin(angk).astype(bf),
            "perm": pm, "ident": ident, "onesc": onesc, "onesr": onesr,
            "wqn": qnw, "wkn": knw,
        })
    return in_maps


def kernel(**inputs):
    in_maps = _make_in_maps(inputs)
    nc = get_nc()
    res = bass_utils.run_bass_kernel_spmd(nc, in_maps, core_ids=list(range(8)))
    outs = [res.results[c]["out"] for c in range(8)]
    full = np.stack([
        outs[0] + outs[1] + outs[2] + outs[3],
        outs[4] + outs[5] + outs[6] + outs[7],
    ]).astype(np.float32)
    return full



# revision 4
# speedup vs baseline: 1.3591x; 1.3591x over previous
"""DFlashAttention Trainium2 kernel (8 NeuronCores), v2.

Sharding: batch (2) data-parallel x kv-head-group (4) tensor-parallel = 8 cores.
Core c: batch b=c//4, kv head g=c%4, q heads [4g..4g+4).

Device pipeline per core (software-pipelined over 9 kv blocks of 512):
  - Q/K/V projections in fp8e4 DoubleRow (2 contraction subtiles per matmul,
    0.5 cyc/row) with host-side hi/lo error compensation (3-term: hi*hi +
    lo_w*hi_x + hi_w*lo_x), weights pre-scaled x256 into e4m3's normal range.
  - K RMSNorm folded into the softmax exp's per-partition scale AP: scores are
    computed on unnormalized rope(K); per-key 1/rms factors (x SCALE/256) come
    from 4 single-column PE matmuls + ln/exp on ACT ([128,4] tiles).
  - RoPE rotate-half via DVE stream_shuffle with head-dims host-interleaved
    (j, 64+j) adjacent so the rotation stays inside 32-partition quadrants;
    rotation sign and q/k norm weights folded into host cos/sin tables.
  - V transposed via DMA xbar (dma_start_transpose) instead of PE.
  - All reciprocals/rsqrts as exp(-ln(x)) so ACT keeps one table loaded.
  - o_proj in bf16 on unnormalized accumulators; per-head softmax denominators
    applied via PE-broadcast reciprocal row, m-outer loop overlaps out DMA.
  - Projection DoubleRow matmuls of block b+2 interleaved between the
    score/attn matmuls of block b so PE never idles on ACT exp latency.
"""

import numpy as np
import ml_dtypes

import concourse.bass as bass
import concourse.mybir as mybir
from concourse import bacc
from concourse.tile import TileContext
from concourse import bass_utils

F32 = mybir.dt.float32
F32R = mybir.dt.float32r
BF16 = mybir.dt.bfloat16
FP8 = mybir.dt.float8e4
E4M3 = ml_dtypes.float8_e4m3
BF = ml_dtypes.bfloat16
DR = mybir.MatmulPerfMode.DoubleRow
AF = mybir.ActivationFunctionType

B, CTX, DRAFT, D = 2, 4096, 512, 2048
H, KVH, HD = 16, 4, 128
NH = H // KVH            # 4 q heads per core
TOT = CTX + DRAFT        # 4608
BLK = 512
NB = TOT // BLK          # 9 kv blocks
SQ = DRAFT               # 512 queries
ND = D // 128            # 16 contraction chunks
EPS = 1e-6
THETA = 10000.0
SCALE = 1.0 / float(np.sqrt(HD))
WS = 256.0               # fp8 weight pre-scale

_CACHE: dict = {}

# shuffle mask: swap adjacent pairs within each 32-partition quadrant
SWAP_MASK = [i + 1 if i % 2 == 0 else i - 1 for i in range(32)]


def _build_nc(pend_depth=3):
    nc = bacc.Bacc()

    xkv_hi = nc.dram_tensor("xkv_hi", [128, ND * TOT], FP8, kind="ExternalInput")
    xkv_lo = nc.dram_tensor("xkv_lo", [128, ND * TOT], FP8, kind="ExternalInput")
    wq_hi = nc.dram_tensor("wq_hi", [128, ND * 512], FP8, kind="ExternalInput")
    wq_lo = nc.dram_tensor("wq_lo", [128, ND * 512], FP8, kind="ExternalInput")
    wk_hi = nc.dram_tensor("wk_hi", [128, ND * HD], FP8, kind="ExternalInput")
    wk_lo = nc.dram_tensor("wk_lo", [128, ND * HD], FP8, kind="ExternalInput")
    wv_hi = nc.dram_tensor("wv_hi", [128, ND * HD], FP8, kind="ExternalInput")
    wv_lo = nc.dram_tensor("wv_lo", [128, ND * HD], FP8, kind="ExternalInput")
    cosq_d = nc.dram_tensor("cosq", [HD, SQ], BF16, kind="ExternalInput")
    sinq_d = nc.dram_tensor("sinq", [HD, SQ], BF16, kind="ExternalInput")
    cosk_d = nc.dram_tensor("cosk", [HD, TOT], BF16, kind="ExternalInput")
    sink_d = nc.dram_tensor("sink", [HD, TOT], BF16, kind="ExternalInput")
    wo_hi_d = nc.dram_tensor("wo_hi", [128, 4 * NH * 512], FP8, kind="ExternalInput")
    wo_lo_d = nc.dram_tensor("wo_lo", [128, 4 * NH * 512], FP8, kind="ExternalInput")
    onesc_d = nc.dram_tensor("onesc", [128, 1], F32R, kind="ExternalInput")
    onesr_d = nc.dram_tensor("onesr", [1, HD], F32R, kind="ExternalInput")
    out = nc.dram_tensor("out", [SQ, D], BF16, kind="ExternalOutput")

    LN_EPS = EPS
    BIAS_K = float(np.log(SCALE / WS))     # exp bias for per-key scale
    BIAS_Q = float(-np.log(WS))            # exp bias for q norm factor
    BIAS_DEN = 0.0   # otn keeps the x256 V scale; po rescaled at copy-out
    SSQ_SC = 1.0 / (HD * WS * WS)          # ln input scale: mean(k^2) from 256^2*k^2

    with nc.allow_low_precision("fp8/bf16 kernel"), TileContext(nc) as tc:
        with (
            tc.tile_pool(name="const", bufs=1) as cpool,
            tc.tile_pool(name="big", bufs=1) as bpool,
            tc.tile_pool(name="acc", bufs=1) as accp,
            tc.tile_pool(name="x4", bufs=10) as x4pool,    # [128,16,512] fp8 streams
            tc.tile_pool(name="cs", bufs=6) as cspool,     # cos/sin per-block slices
            tc.tile_pool(name="scr", bufs=2) as scr,
            tc.tile_pool(name="pex", bufs=8) as pex,
            tc.tile_pool(name="vnp", bufs=3) as vnp,
            tc.tile_pool(name="ps_a", bufs=2, space="PSUM") as ps_a,   # kt/vt proj
            tc.tile_pool(name="ps_b", bufs=3, space="PSUM") as ps_b,   # st / psq / po
            tc.tile_pool(name="ps_c", bufs=2, space="PSUM") as ps_c,   # ot / misc
            tc.tile_pool(name="ps_d", bufs=1, space="PSUM") as ps_d,   # ssq minis
        ):
            onescb = cpool.tile([128, 1], BF16, name="onescb")
            nc.vector.memset(onescb[:, :], 1.0)
            onesc_r = cpool.tile([128, 1], F32R, name="onesc_r")
            nc.sync.dma_start(onesc_r[:, :], onesc_d[:, :])
            onesr_r = cpool.tile([1, HD], F32R, name="onesr_r")
            nc.sync.dma_start(onesr_r[:, :], onesr_d[:, :])
            eps128 = cpool.tile([128, 1], F32, name="eps128")
            nc.vector.memset(eps128[:, :], LN_EPS)
            biask128 = cpool.tile([128, 1], F32, name="biask128")
            nc.vector.memset(biask128[:, :], BIAS_K)
            eps1 = cpool.tile([1, 1], F32, name="eps1")
            nc.vector.memset(eps1[:, :], LN_EPS)
            biasq1 = cpool.tile([1, 1], F32, name="biasq1")
            nc.vector.memset(biasq1[:, :], BIAS_Q)
            zero1 = cpool.tile([1, 1], F32, name="zero1")
            nc.vector.memset(zero1[:, :], 0.0)
            biasden1 = cpool.tile([1, 1], F32, name="biasden1")
            nc.vector.memset(biasden1[:, :], BIAS_DEN)
            zero128 = cpool.tile([128, 1], F32, name="zero128")
            nc.vector.memset(zero128[:, :], 0.0)

            # Preload the one act table serving both Exp and Ln so the
            # finalize pass doesn't thrash between exp-only/ln-only sets.
            from concourse.hw_specs import get_activation_tables
            tabs = get_activation_tables(nc.m.arch)
            atl_id = next(i for i, (tname, funcs) in enumerate(tabs.items())
                          if AF.Exp in funcs and AF.Ln in funcs)
            nc.scalar.add_instruction(mybir.InstLoadActFuncSet(
                name=nc.get_next_instruction_name(), ins=[], outs=[],
                act_func_set_id=atl_id))

            qrope = [accp.tile([HD, SQ], BF16, name=f"qrope{h}") for h in range(NH)]
            otsb = [accp.tile([HD, SQ], F32, name=f"otsb{h}") for h in range(NH)]
            pacc = [accp.tile([128, SQ], F32R, name=f"pacc{h}") for h in range(NH)]
            otn_hi = accp.tile([HD, NH, SQ], FP8, name="otn_hi")
            otn_lo = accp.tile([HD, NH, SQ], FP8, name="otn_lo")

            # ---------- phase 1: DMAs ----------
            def load16(dst_pool, name, src, csl):
                t = dst_pool.tile([128, ND, BLK], FP8, name=name, tag="x4")
                nc.sync.dma_start(
                    t[:, :, :],
                    src[:, :].rearrange("p (d c) -> p d c", d=ND)[:, :, csl])
                return t

            # Q-phase operands: quarter-granular DMAs, wq interleaved with the
            # draft slice of the kv stream (block 8 doubles as the Q input).
            def alloc16(name):
                return x4pool.tile([128, ND, BLK], FP8, name=name, tag="x4")

            def dma_quarter(t, src, qi, csl):
                sl = slice(4 * qi, 4 * qi + 4)
                nc.sync.dma_start(
                    t[:, sl, :],
                    src[:, :].rearrange("p (d c) -> p d c", d=ND)[:, sl, csl])

            dsl_draft = slice(CTX, TOT)
            full_sl = slice(0, BLK)
            wqh_t, xh8_t = alloc16("wqh"), alloc16("xh8")
            wql_t, xl8_t = alloc16("wql"), alloc16("xl8")

            for qi in range(4):
                dma_quarter(wqh_t, wq_hi, qi, full_sl)
                dma_quarter(xh8_t, xkv_hi, qi, dsl_draft)
            for qi in range(4):
                dma_quarter(wql_t, wq_lo, qi, full_sl)
                dma_quarter(xl8_t, xkv_lo, qi, dsl_draft)
            wkh = bpool.tile([128, ND, HD], FP8, name="wkh")
            nc.sync.dma_start(wkh[:, :, :], wk_hi[:, :].rearrange("p (d c) -> p d c", d=ND))
            wvh = bpool.tile([128, ND, HD], FP8, name="wvh")
            nc.sync.dma_start(wvh[:, :, :], wv_hi[:, :].rearrange("p (d c) -> p d c", d=ND))
            wkl = bpool.tile([128, ND, HD], FP8, name="wkl")
            nc.sync.dma_start(wkl[:, :, :], wk_lo[:, :].rearrange("p (d c) -> p d c", d=ND))
            wvl = bpool.tile([128, ND, HD], FP8, name="wvl")
            nc.sync.dma_start(wvl[:, :, :], wv_lo[:, :].rearrange("p (d c) -> p d c", d=ND))
            cosq = bpool.tile([HD, SQ], BF16, name="cosq_sb")
            nc.sync.dma_start(cosq[:, :], cosq_d[:, :])
            sinq = bpool.tile([HD, SQ], BF16, name="sinq_sb")
            nc.sync.dma_start(sinq[:, :], sinq_d[:, :])

            state: dict = {}

            def load_cs(cb):
                csl = slice(cb * BLK, (cb + 1) * BLK)
                ck = cspool.tile([HD, BLK], BF16, name=f"ck{cb}", tag="cs")
                nc.sync.dma_start(ck[:, :], cosk_d[:, csl])
                sk = cspool.tile([HD, BLK], BF16, name=f"sk{cb}", tag="cs")
                nc.sync.dma_start(sk[:, :], sink_d[:, csl])
                state[("cs", cb)] = (ck, sk)

            def load_block(cb):
                csl = slice(cb * BLK, (cb + 1) * BLK)
                xh = load16(x4pool, f"xh{cb}", xkv_hi, csl)
                xl = load16(x4pool, f"xl{cb}", xkv_lo, csl)
                state[("x", cb)] = (xh, xl)
                load_cs(cb)

            # 3-term fp8 DoubleRow projection matmuls; returns list of closures
            def dr_emitters(out_ps, whi_t, wlo_t, xhi_t, xlo_t, colsl):
                ems = []
                terms = [(whi_t, xhi_t), (wlo_t, xhi_t), (whi_t, xlo_t)]
                n_tot = 3 * (ND // 2)
                k = [0]

                def mk(i):
                    def em():
                        ti, dp = divmod(i, ND // 2)
                        w_t, x_t = terms[ti]
                        sl = slice(2 * dp, 2 * dp + 2)
                        nc.tensor.matmul(out_ps[:, :], w_t[:, sl, colsl],
                                         x_t[:, sl, :],
                                         start=(i == 0), stop=(i == n_tot - 1),
                                         perf_mode=DR, skip_group_check=True)
                    return em
                # order: iterate dp-major inside each term for locality
                for ti in range(3):
                    for dp in range(ND // 2):
                        ems.append(mk(ti * (ND // 2) + dp))
                return ems

            def proj_emitters(cb):
                """Returns (kt_ps, vt_ps, [closures]) for block cb."""
                xh, xl = state.pop(("x", cb))
                kt = ps_a.tile([HD, BLK], F32, name=f"kt{cb}", tag="proj")
                vt = ps_a.tile([HD, BLK], F32, name=f"vt{cb}", tag="proj")
                ems = dr_emitters(kt, wkh, wkl, xh, xl, slice(None))
                ems += dr_emitters(vt, wvh, wvl, xh, xl, slice(None))
                state[("ktvt", cb)] = (kt, vt)
                return ems

            def prep_block(cb):
                """Norm-stats + rope for K, transpose V (block cb)."""
                kt, vt = state.pop(("ktvt", cb))
                ck, sk = state.pop(("cs", cb))
                src16 = scr.tile([128, BLK], BF16, name=f"src{cb}", tag="src")
                nc.vector.tensor_copy(src16[:, :], kt[:, :])
                sq = scr.tile([128, BLK], BF16, name=f"sq{cb}", tag="sq")
                nc.gpsimd.tensor_mul(sq[:, :], src16[:, :], src16[:, :])
                ssq = ps_d.tile([128, 4], F32, name=f"ssq{cb}", tag="ssq")
                for j in range(4):
                    nc.tensor.matmul(ssq[:, j:j + 1],
                                     sq[:, j * 128:(j + 1) * 128],
                                     onescb[:, :], start=True, stop=True)
                u = scr.tile([128, 4], F32, name=f"u{cb}", tag="u")
                nc.scalar.activation(u[:, :], ssq[:, :], AF.Ln,
                                     bias=eps128[:, :], scale=SSQ_SC)
                a_k = scr.tile([128, 4], F32, name=f"ak{cb}", tag="ak", bufs=3)
                nc.scalar.activation(a_k[:, :], u[:, :], AF.Exp,
                                     bias=biask128[:, :], scale=-0.5)
                sh = scr.tile([128, BLK], BF16, name=f"sh{cb}", tag="sh")
                nc.vector.stream_shuffle(sh[:, :], src16[:, :], SWAP_MASK)
                t1 = scr.tile([128, BLK], BF16, name=f"t1{cb}", tag="t1")
                nc.gpsimd.tensor_mul(t1[:, :], src16[:, :], ck[:, :])
                t2 = scr.tile([128, BLK], BF16, name=f"t2{cb}", tag="t2")
                nc.vector.tensor_mul(t2[:, :], sh[:, :], sk[:, :])
                ktf = scr.tile([128, BLK], BF16, name=f"ktf{cb}", tag="ktf", bufs=3)
                nc.vector.tensor_add(ktf[:, :], t1[:, :], t2[:, :])
                # V: bf16 copy + xbar transpose
                vt16 = scr.tile([HD, BLK], BF16, name=f"vt16{cb}", tag="vt16")
                nc.vector.tensor_copy(vt16[:, :], vt[:, :])
                vn = vnp.tile([128, 4, HD], BF16, name=f"vn{cb}", tag="vn")
                nc.sync.dma_start_transpose(vn[:, :, :], vt16[:, :])
                state[("ktf", cb)] = ktf
                state[("ak", cb)] = a_k
                state[("vn", cb)] = vn

            def attn_block(cb, fillers, first=False, last=False):
                ktf = state.pop(("ktf", cb))
                a_k = state.pop(("ak", cb))
                vn = state.pop(("vn", cb))
                fi = [0]

                def fill(n):
                    for _ in range(n):
                        if fi[0] < len(fillers):
                            fillers[fi[0]]()
                            fi[0] += 1

                pend = []

                def normalize_head(h):
                    den = ps_d.tile([1, SQ], F32, name=f"den{h}", tag="ssq")
                    nc.tensor.matmul(den[:, :], onesc_r[:, :], pacc[h][:, :],
                                     start=True, stop=True, skip_group_check=True)
                    ud = scr.tile([1, SQ], F32, name=f"ud{h}", tag="u")
                    nc.scalar.activation(ud[:, :], den[:, :], AF.Ln,
                                         bias=zero1[:, :], scale=1.0)
                    rd = scr.tile([1, SQ], F32R, name=f"rd{h}", tag="rdt")
                    nc.scalar.activation(rd[:, :], ud[:, :], AF.Exp,
                                         bias=biasden1[:, :], scale=-1.0)
                    nfd = ps_c.tile([HD, SQ], F32, name=f"nfd{h}", tag="ot")
                    nc.tensor.matmul(nfd[:, :], onesr_r[:, :], rd[:, :],
                                     start=True, stop=True, skip_group_check=True)
                    t_n = scr.tile([HD, SQ], F32, name=f"tn{h}", tag="tn")
                    nc.vector.tensor_mul(t_n[:, :], otsb[h][:, :], nfd[:, :])
                    nc.scalar.copy(otn_hi[:, h, :], t_n[:, :])
                    if h % 2 == 0:
                        nc.gpsimd.tensor_sub(otn_lo[:, h, :], t_n[:, :], otn_hi[:, h, :])
                    else:
                        nc.vector.tensor_sub(otn_lo[:, h, :], t_n[:, :], otn_hi[:, h, :])

                def flush_one():
                    h, j, p_t, ot_ps = pend.pop(0)
                    nc.tensor.matmul(ot_ps[:, :], vn[:, j, :], p_t[:, :],
                                     start=(j == 0), stop=(j == 3),
                                     skip_group_check=True)
                    if first and j == 0:
                        nc.gpsimd.tensor_copy(pacc[h][:, :], p_t[:, :])
                    else:
                        nc.gpsimd.tensor_add(pacc[h][:, :], pacc[h][:, :], p_t[:, :])
                    if j == 3:
                        if first:
                            nc.vector.tensor_copy(otsb[h][:, :], ot_ps[:, :])
                        else:
                            nc.vector.tensor_add(otsb[h][:, :], otsb[h][:, :], ot_ps[:, :])
                    if last and j == 1 and h >= 1:
                        normalize_head(h - 1)

                for h in range(NH):
                    ot_ps = ps_c.tile([HD, SQ], F32, name=f"ot{cb}_{h}", tag="ot")
                    for j in range(4):
                        st = ps_b.tile([128, SQ], F32, name=f"st{cb}_{h}_{j}", tag="st")
                        nc.tensor.matmul(st[:, :], ktf[:, j * 128:(j + 1) * 128],
                                         qrope[h][:, :], start=True, stop=True,
                                         skip_group_check=True)
                        p_t = pex.tile([128, SQ], BF16, name=f"p{cb}_{h}_{j}", tag="p")
                        nc.scalar.activation(p_t[:, :], st[:, :], AF.Exp,
                                             bias=zero128[:, :],
                                             scale=a_k[:, j:j + 1])
                        pend.append((h, j, p_t, ot_ps))
                        fill(3)
                        if len(pend) >= pend_depth:
                            flush_one()
                while pend:
                    flush_one()
                fill(len(fillers))
                if last:
                    normalize_head(3)

            # ---------- phase 1: Q projection + norm + rope ----------
            # dp-major across heads: matmuls fire as DMA quarters land
            psqs = [(ps_b if h < 2 else ps_a).tile([HD, SQ], F32, name=f"psq{h}",
                                                   tag="st" if h < 2 else "proj")
                    for h in range(NH)]
            state[("x", 8)] = (xh8_t, xl8_t)
            load_cs(8)
            qterms = [(wqh_t, xh8_t), (wql_t, xh8_t), (wqh_t, xl8_t)]
            for ti, (w_t, x_t) in enumerate(qterms):
                for dp in range(ND // 2):
                    dsl = slice(2 * dp, 2 * dp + 2)
                    for h in range(NH):
                        hsl = slice(h * HD, (h + 1) * HD)
                        nc.tensor.matmul(
                            psqs[h][:, :], w_t[:, dsl, hsl], x_t[:, dsl, :],
                            start=(ti == 0 and dp == 0),
                            stop=(ti == 2 and dp == ND // 2 - 1),
                            perf_mode=DR, skip_group_check=True)
            for h in range(NH):
                psq = psqs[h]
                srcq = scr.tile([HD, SQ], F32R, name=f"srcq{h}", tag="src")
                nc.vector.tensor_copy(srcq[:, :], psq[:, :])
                sqq = scr.tile([HD, SQ], BF16, name=f"sqq{h}", tag="sq")
                nc.gpsimd.tensor_mul(sqq[:, :], srcq[:, :], srcq[:, :])
                ssqq = ps_c.tile([1, SQ], F32, name=f"ssqq{h}", tag="ot")
                nc.tensor.matmul(ssqq[:, :], onescb[:, :], sqq[:, :],
                                 start=True, stop=True)
                uq = scr.tile([1, SQ], F32, name=f"uq{h}", tag="u")
                nc.scalar.activation(uq[:, :], ssqq[:, :], AF.Ln,
                                     bias=eps1[:, :], scale=SSQ_SC)
                rsq = scr.tile([1, SQ], F32R, name=f"rsq{h}", tag="rsy")
                nc.scalar.activation(rsq[:, :], uq[:, :], AF.Exp,
                                     bias=biasq1[:, :], scale=-0.5)
                nfq = ps_c.tile([HD, SQ], F32, name=f"nfq{h}", tag="ot")
                nc.tensor.matmul(nfq[:, :], onesr_r[:, :], rsq[:, :],
                                 start=True, stop=True)
                xnq = scr.tile([HD, SQ], BF16, name=f"xnq{h}", tag="t1")
                nc.vector.tensor_mul(xnq[:, :], srcq[:, :], nfq[:, :])
                shq = scr.tile([HD, SQ], BF16, name=f"shq{h}", tag="sh")
                nc.vector.stream_shuffle(shq[:, :], xnq[:, :], SWAP_MASK)
                tq1 = scr.tile([HD, SQ], BF16, name=f"tq1{h}", tag="t2")
                nc.gpsimd.tensor_mul(tq1[:, :], xnq[:, :], cosq[:, :])
                tq2 = scr.tile([HD, SQ], BF16, name=f"tq2{h}", tag="vt16")
                nc.vector.tensor_mul(tq2[:, :], shq[:, :], sinq[:, :])
                nc.vector.tensor_add(qrope[h][:, :], tq1[:, :], tq2[:, :])

            # ---------- phase 2: kv block pipeline (block 8 first) ----------
            SEQ = [8] + list(range(NB - 1))
            load_block(0)
            load_block(1)
            for em in proj_emitters(8):
                em()
            prep_block(8)
            for em in proj_emitters(0):
                em()
            prep_block(0)
            for s in range(NB):
                bid = SEQ[s]
                if s + 3 < NB:
                    load_block(SEQ[s + 3])
                if s == NB - 2:
                    wo8h = bpool.tile([128, 4, NH, 512], FP8, name="wo8h")
                    nc.sync.dma_start(
                        wo8h[:, :, :, :],
                        wo_hi_d[:, :].rearrange("p (n h c) -> p n h c", n=4, h=NH))
                    wo8l = bpool.tile([128, 4, NH, 512], FP8, name="wo8l")
                    nc.sync.dma_start(
                        wo8l[:, :, :, :],
                        wo_lo_d[:, :].rearrange("p (n h c) -> p n h c", n=4, h=NH))
                fillers = proj_emitters(SEQ[s + 2]) if s + 2 < NB else []
                attn_block(bid, fillers, first=(s == 0), last=(s == NB - 1))
                if s + 2 < NB:
                    prep_block(SEQ[s + 2])

            # ---------- phase 3: o_proj ----------
            osbm = [scr.tile([128, D], BF16, name=f"osbm{m}", tag=f"osbm{m}", bufs=1)
                    for m in range(4)]
            for m in range(4):
                msl = slice(m * HD, (m + 1) * HD)
                for n in range(4):
                    po = ps_b.tile([128, 512], F32, name=f"po{m}_{n}", tag="st")
                    pterms = [(otn_hi, wo8h), (otn_lo, wo8h), (otn_hi, wo8l)]
                    for ti, (o_t, w_t) in enumerate(pterms):
                        for hp in range(2):
                            hsl2 = slice(2 * hp, 2 * hp + 2)
                            nc.tensor.matmul(po[:, :], o_t[:, hsl2, msl],
                                             w_t[:, n, hsl2, :],
                                             start=(ti == 0 and hp == 0),
                                             stop=(ti == 2 and hp == 1),
                                             perf_mode=DR, skip_group_check=True)
                    nsl = slice(n * 512, (n + 1) * 512)
                    RS = 1.0 / (WS * WS)
                    if (m * 4 + n) % 2 == 0:
                        nc.vector.tensor_scalar_mul(osbm[m][:, nsl], po[:, :], RS)
                    else:
                        nc.scalar.mul(osbm[m][:, nsl], po[:, :], RS)
                    nc.sync.dma_start(out[m * 128:(m + 1) * 128, nsl], osbm[m][:, nsl])
    nc.finalize()
    return nc


def get_nc(**kw):
    key = ("nc2", tuple(sorted(kw.items())))
    if key not in _CACHE:
        _CACHE[key] = _build_nc(**kw)
    return _CACHE[key]


# ---------------- host-side data prep ----------------

INTL = np.empty(HD, np.int64)   # new slot -> original dim
INTL[0::2] = np.arange(64)
INTL[1::2] = np.arange(64) + 64


def _hi_lo(a32):
    hi = a32.astype(E4M3)
    lo = (a32 - hi.astype(np.float32)).astype(E4M3)
    return hi, lo


def _pack16(a):
    """[D, C] -> [128, ND*C] partition-major chunk layout."""
    Dd, C = a.shape
    return np.ascontiguousarray(
        a.reshape(ND, 128, C).transpose(1, 0, 2).reshape(128, ND * C))


def _rope_tables(pos, w):
    """pos [S], w [HD] -> cos/sin tables [HD, S] interleaved, w- and sign-folded."""
    inv = 1.0 / (THETA ** (np.arange(0, HD, 2, dtype=np.float32) / np.float32(HD)))
    ang = inv[:, None] * pos[None, :].astype(np.float32)   # [64, S]
    c, s = np.cos(ang), np.sin(ang)
    cosT = np.empty((HD, len(pos)), np.float32)
    sinT = np.empty((HD, len(pos)), np.float32)
    cosT[0::2] = w[:64, None] * c
    cosT[1::2] = w[64:, None] * c
    sinT[0::2] = -w[64:, None] * s   # partner dim's weight rides the sin term
    sinT[1::2] = w[:64, None] * s
    return cosT.astype(BF), sinT.astype(BF)


def _make_in_maps(inputs):
    draft = np.asarray(inputs["draft_hidden"], np.float32)
    ctx = np.asarray(inputs["context_hidden"], np.float32)
    Wq = np.asarray(inputs["Wq"], np.float32)
    Wk = np.asarray(inputs["Wk"], np.float32)
    Wv = np.asarray(inputs["Wv"], np.float32)
    Wo = np.asarray(inputs["Wo"], np.float32)
    qnw = np.asarray(inputs["q_norm_w"], np.float32).reshape(HD)
    knw = np.asarray(inputs["k_norm_w"], np.float32).reshape(HD)
    cpos = np.asarray(inputs["context_position_ids"])
    dpos = np.asarray(inputs["draft_position_ids"])

    in_maps = []
    xkv_c = {}
    xd_c = {}
    for c in range(8):
        b, g = c // 4, c % 4
        if b not in xkv_c:
            kvin = np.concatenate([ctx[b], draft[b]], axis=0)        # [TOT, D]
            xh, xl = _hi_lo(kvin.T)                                  # [D, TOT]
            xkv_c[b] = (_pack16(xh), _pack16(xl))
        xkv_hi, xkv_lo = xkv_c[b]

        # Wq for this core's 4 heads, head-dim interleaved, scaled, transposed
        Wqc = Wq.reshape(H, HD, D)[4 * g:4 * g + 4][:, INTL, :]      # [4,128,D]
        WqT = Wqc.reshape(4 * HD, D).T * WS                          # [D, 512]
        wqh, wql = _hi_lo(WqT)
        Wkc = Wk.reshape(KVH, HD, D)[g][INTL, :]                     # [128, D]
        WkT = Wkc.T * WS
        wkh, wkl = _hi_lo(WkT)
        WvT = Wv.reshape(KVH, HD, D)[g].T * WS                       # [D, 128]
        wvh, wvl = _hi_lo(WvT)

        fpos = np.concatenate([cpos[b], dpos[b]])
        cosk, sink = _rope_tables(fpos, knw)
        cosq, sinq = _rope_tables(dpos[b], qnw)

        # wo: [128p(hd), n, h, c] = Wo[n*512+c, (4g+h)*128+p]
        Woc = Wo[:, 4 * g * HD:(4 * g + 4) * HD]                     # [D, 512]
        woN = Woc.reshape(4, 512, NH, HD).transpose(3, 0, 2, 1)      # [128, 4n, h, 512c]
        woN = np.ascontiguousarray(woN.reshape(128, -1)) * WS
        wo_hi, wo_lo = _hi_lo(woN)

        in_maps.append({
            "xkv_hi": xkv_hi, "xkv_lo": xkv_lo,
            "wq_hi": _pack16(wqh), "wq_lo": _pack16(wql),
            "wk_hi": _pack16(wkh), "wk_lo": _pack16(wkl),
            "wv_hi": _pack16(wvh), "wv_lo": _pack16(wvl),
            "cosq": cosq, "sinq": sinq, "cosk": cosk, "sink": sink,
            "wo_hi": wo_hi, "wo_lo": wo_lo,
            "onesc": np.ones((128, 1), np.float32),
            "onesr": np.ones((1, HD), np.float32),
        })
    return in_maps


def kernel(**inputs):
    in_maps = _make_in_maps(inputs)
    nc = get_nc()
    res = bass_utils.run_bass_kernel_spmd(nc, in_maps, core_ids=list(range(8)))
    outs = [np.asarray(res.results[c]["out"], np.float32) for c in range(8)]
    full = np.stack([
        outs[0] + outs[1] + outs[2] + outs[3],
        outs[4] + outs[5] + outs[6] + outs[7],
    ]).astype(np.float32)
    return full


# revision 7
# speedup vs baseline: 1.3911x; 1.0235x over previous
"""DFlashAttention Trainium2 kernel (8 NeuronCores), v2.

Sharding: batch (2) data-parallel x kv-head-group (4) tensor-parallel = 8 cores.
Core c: batch b=c//4, kv head g=c%4, q heads [4g..4g+4).

Device pipeline per core (software-pipelined over 9 kv blocks of 512):
  - Q/K/V projections in fp8e4 DoubleRow (2 contraction subtiles per matmul,
    0.5 cyc/row) with host-side hi/lo error compensation (3-term: hi*hi +
    lo_w*hi_x + hi_w*lo_x), weights pre-scaled x256 into e4m3's normal range.
  - K RMSNorm folded into the softmax exp's per-partition scale AP: scores are
    computed on unnormalized rope(K); per-key 1/rms factors (x SCALE/256) come
    from 4 single-column PE matmuls + ln/exp on ACT ([128,4] tiles).
  - RoPE rotate-half via DVE stream_shuffle with head-dims host-interleaved
    (j, 64+j) adjacent so the rotation stays inside 32-partition quadrants;
    rotation sign and q/k norm weights folded into host cos/sin tables.
  - V transposed via DMA xbar (dma_start_transpose) instead of PE.
  - All reciprocals/rsqrts as exp(-ln(x)) so ACT keeps one table loaded.
  - o_proj in bf16 on unnormalized accumulators; per-head softmax denominators
    applied via PE-broadcast reciprocal row, m-outer loop overlaps out DMA.
  - Projection DoubleRow matmuls of block b+2 interleaved between the
    score/attn matmuls of block b so PE never idles on ACT exp latency.
"""

import numpy as np
import ml_dtypes

import concourse.bass as bass
import concourse.mybir as mybir
from concourse import bacc
from concourse.tile import TileContext
from concourse import bass_utils

F32 = mybir.dt.float32
F32R = mybir.dt.float32r
BF16 = mybir.dt.bfloat16
FP8 = mybir.dt.float8e4
E4M3 = ml_dtypes.float8_e4m3
BF = ml_dtypes.bfloat16
DR = mybir.MatmulPerfMode.DoubleRow
AF = mybir.ActivationFunctionType

B, CTX, DRAFT, D = 2, 4096, 512, 2048
H, KVH, HD = 16, 4, 128
NH = H // KVH            # 4 q heads per core
TOT = CTX + DRAFT        # 4608
BLK = 512
NB = TOT // BLK          # 9 kv blocks
SQ = DRAFT               # 512 queries
ND = D // 128            # 16 contraction chunks
EPS = 1e-6
THETA = 10000.0
SCALE = 1.0 / float(np.sqrt(HD))
WS = 256.0               # fp8 weight pre-scale

_CACHE: dict = {}

# shuffle mask: swap adjacent pairs within each 32-partition quadrant
SWAP_MASK = [i + 1 if i % 2 == 0 else i - 1 for i in range(32)]


def _build_nc(pend_depth=3):
    nc = bacc.Bacc()

    xkv_hi = nc.dram_tensor("xkv_hi", [128, ND * TOT], FP8, kind="ExternalInput")
    xkv_lo = nc.dram_tensor("xkv_lo", [128, ND * TOT], FP8, kind="ExternalInput")
    wq_hi = nc.dram_tensor("wq_hi", [128, ND * 512], FP8, kind="ExternalInput")
    wq_lo = nc.dram_tensor("wq_lo", [128, ND * 512], FP8, kind="ExternalInput")
    wk_hi = nc.dram_tensor("wk_hi", [128, ND * HD], FP8, kind="ExternalInput")
    wk_lo = nc.dram_tensor("wk_lo", [128, ND * HD], FP8, kind="ExternalInput")
    wv_hi = nc.dram_tensor("wv_hi", [128, ND * HD], FP8, kind="ExternalInput")
    wv_lo = nc.dram_tensor("wv_lo", [128, ND * HD], FP8, kind="ExternalInput")
    cosq_d = nc.dram_tensor("cosq", [HD, SQ], BF16, kind="ExternalInput")
    sinq_d = nc.dram_tensor("sinq", [HD, SQ], BF16, kind="ExternalInput")
    cosk_d = nc.dram_tensor("cosk", [HD, TOT], BF16, kind="ExternalInput")
    sink_d = nc.dram_tensor("sink", [HD, TOT], BF16, kind="ExternalInput")
    wo_hi_d = nc.dram_tensor("wo_hi", [128, 4 * NH * 512], FP8, kind="ExternalInput")
    wo_lo_d = nc.dram_tensor("wo_lo", [128, 4 * NH * 512], FP8, kind="ExternalInput")
    onesc_d = nc.dram_tensor("onesc", [128, 1], F32R, kind="ExternalInput")
    onesr_d = nc.dram_tensor("onesr", [1, HD], F32R, kind="ExternalInput")
    out = nc.dram_tensor("out", [SQ, D], BF16, kind="ExternalOutput")

    LN_EPS = EPS
    BIAS_K = float(np.log(SCALE / WS))     # exp bias for per-key scale
    BIAS_Q = float(-np.log(WS))            # exp bias for q norm factor
    BIAS_DEN = 0.0   # otn keeps the x256 V scale; po rescaled at copy-out
    SSQ_SC = 1.0 / (HD * WS * WS)          # ln input scale: mean(k^2) from 256^2*k^2

    with nc.allow_low_precision("fp8/bf16 kernel"), TileContext(nc) as tc:
        with (
            tc.tile_pool(name="const", bufs=1) as cpool,
            tc.tile_pool(name="big", bufs=1) as bpool,
            tc.tile_pool(name="acc", bufs=1) as accp,
            tc.tile_pool(name="x4", bufs=10) as x4pool,    # [128,16,512] fp8 streams
            tc.tile_pool(name="cs", bufs=6) as cspool,     # cos/sin per-block slices
            tc.tile_pool(name="scr", bufs=2) as scr,
            tc.tile_pool(name="pex", bufs=8) as pex,
            tc.tile_pool(name="vnp", bufs=3) as vnp,
            tc.tile_pool(name="ps_a", bufs=2, space="PSUM") as ps_a,   # kt/vt proj
            tc.tile_pool(name="ps_b", bufs=3, space="PSUM") as ps_b,   # st / psq / po
            tc.tile_pool(name="ps_c", bufs=2, space="PSUM") as ps_c,   # ot / misc
            tc.tile_pool(name="ps_d", bufs=1, space="PSUM") as ps_d,   # ssq minis
        ):
            onescb = cpool.tile([128, 1], BF16, name="onescb")
            nc.vector.memset(onescb[:, :], 1.0)
            onesc_r = cpool.tile([128, 1], F32R, name="onesc_r")
            nc.sync.dma_start(onesc_r[:, :], onesc_d[:, :])
            onesr_r = cpool.tile([1, HD], F32R, name="onesr_r")
            nc.sync.dma_start(onesr_r[:, :], onesr_d[:, :])
            eps128 = cpool.tile([128, 1], F32, name="eps128")
            nc.vector.memset(eps128[:, :], LN_EPS)
            biask128 = cpool.tile([128, 1], F32, name="biask128")
            nc.vector.memset(biask128[:, :], BIAS_K)
            eps1 = cpool.tile([1, 1], F32, name="eps1")
            nc.vector.memset(eps1[:, :], LN_EPS)
            biasq1 = cpool.tile([1, 1], F32, name="biasq1")
            nc.vector.memset(biasq1[:, :], BIAS_Q)
            zero1 = cpool.tile([1, 1], F32, name="zero1")
            nc.vector.memset(zero1[:, :], 0.0)
            biasden1 = cpool.tile([1, 1], F32, name="biasden1")
            nc.vector.memset(biasden1[:, :], BIAS_DEN)
            zero128 = cpool.tile([128, 1], F32, name="zero128")
            nc.vector.memset(zero128[:, :], 0.0)

            # Preload the one act table serving both Exp and Ln so the
            # finalize pass doesn't thrash between exp-only/ln-only sets.
            from concourse.hw_specs import get_activation_tables
            tabs = get_activation_tables(nc.m.arch)
            atl_id = next(i for i, (tname, funcs) in enumerate(tabs.items())
                          if AF.Exp in funcs and AF.Ln in funcs)
            nc.scalar.add_instruction(mybir.InstLoadActFuncSet(
                name=nc.get_next_instruction_name(), ins=[], outs=[],
                act_func_set_id=atl_id))

            qrope = [accp.tile([HD, SQ], BF16, name=f"qrope{h}") for h in range(NH)]
            otsb = [accp.tile([HD, SQ], F32, name=f"otsb{h}") for h in range(NH)]
            pacc = [accp.tile([128, SQ], F32R, name=f"pacc{h}") for h in range(NH)]
            otn_hi = accp.tile([HD, NH, SQ], FP8, name="otn_hi")
            otn_lo = accp.tile([HD, NH, SQ], FP8, name="otn_lo")

            # ---------- phase 1: DMAs ----------
            def load16(dst_pool, name, src, csl, eng=None):
                t = dst_pool.tile([128, ND, BLK], FP8, name=name, tag="x4")
                (eng or nc.sync).dma_start(
                    t[:, :, :],
                    src[:, :].rearrange("p (d c) -> p d c", d=ND)[:, :, csl])
                return t

            # Q-phase operands: quarter-granular DMAs, wq interleaved with the
            # draft slice of the kv stream (block 8 doubles as the Q input).
            def alloc16(name):
                return x4pool.tile([128, ND, BLK], FP8, name=name, tag="x4")

            def dma_quarter(t, src, qi, csl, eng=None):
                sl = slice(4 * qi, 4 * qi + 4)
                (eng or nc.sync).dma_start(
                    t[:, sl, :],
                    src[:, :].rearrange("p (d c) -> p d c", d=ND)[:, sl, csl])

            dsl_draft = slice(CTX, TOT)
            full_sl = slice(0, BLK)
            wqh_t, xh8_t = alloc16("wqh"), alloc16("xh8")
            wql_t, xl8_t = alloc16("wql"), alloc16("xl8")

            def dma_eighth(t, src, ei, csl, eng=None):
                sl = slice(2 * ei, 2 * ei + 2)
                (eng or nc.sync).dma_start(
                    t[:, sl, :],
                    src[:, :].rearrange("p (d c) -> p d c", d=ND)[:, sl, csl])

            dma_eighth(wqh_t, wq_hi, 0, full_sl)
            dma_eighth(xh8_t, xkv_hi, 0, dsl_draft, eng=nc.scalar)
            dma_eighth(wqh_t, wq_hi, 1, full_sl)
            dma_eighth(xh8_t, xkv_hi, 1, dsl_draft, eng=nc.scalar)
            for qi in range(1, 4):
                dma_quarter(wqh_t, wq_hi, qi, full_sl)
                dma_quarter(xh8_t, xkv_hi, qi, dsl_draft, eng=nc.scalar)
            for qi in range(4):
                dma_quarter(wql_t, wq_lo, qi, full_sl)
                dma_quarter(xl8_t, xkv_lo, qi, dsl_draft, eng=nc.scalar)
            wkh = bpool.tile([128, ND, HD], FP8, name="wkh")
            nc.sync.dma_start(wkh[:, :, :], wk_hi[:, :].rearrange("p (d c) -> p d c", d=ND))
            wvh = bpool.tile([128, ND, HD], FP8, name="wvh")
            nc.scalar.dma_start(wvh[:, :, :], wv_hi[:, :].rearrange("p (d c) -> p d c", d=ND))
            wkl = bpool.tile([128, ND, HD], FP8, name="wkl")
            nc.sync.dma_start(wkl[:, :, :], wk_lo[:, :].rearrange("p (d c) -> p d c", d=ND))
            wvl = bpool.tile([128, ND, HD], FP8, name="wvl")
            nc.scalar.dma_start(wvl[:, :, :], wv_lo[:, :].rearrange("p (d c) -> p d c", d=ND))
            cosq = bpool.tile([HD, SQ], BF16, name="cosq_sb")
            nc.scalar.dma_start(cosq[:, :], cosq_d[:, :])
            sinq = bpool.tile([HD, SQ], BF16, name="sinq_sb")
            nc.scalar.dma_start(sinq[:, :], sinq_d[:, :])

            state: dict = {}

            def load_cs(cb):
                csl = slice(cb * BLK, (cb + 1) * BLK)
                ck = cspool.tile([HD, BLK], BF16, name=f"ck{cb}", tag="cs")
                nc.sync.dma_start(ck[:, :], cosk_d[:, csl])
                sk = cspool.tile([HD, BLK], BF16, name=f"sk{cb}", tag="cs")
                nc.sync.dma_start(sk[:, :], sink_d[:, csl])
                state[("cs", cb)] = (ck, sk)

            def load_block(cb, split=False):
                csl = slice(cb * BLK, (cb + 1) * BLK)
                xh = load16(x4pool, f"xh{cb}", xkv_hi, csl)
                xl = load16(x4pool, f"xl{cb}", xkv_lo, csl, eng=nc.scalar if split else None)
                state[("x", cb)] = (xh, xl)
                load_cs(cb)

            # 3-term fp8 DoubleRow projection matmuls; returns list of closures
            def dr_emitters(out_ps, whi_t, wlo_t, xhi_t, xlo_t, colsl):
                ems = []
                terms = [(whi_t, xhi_t), (wlo_t, xhi_t), (whi_t, xlo_t)]
                n_tot = 3 * (ND // 2)
                k = [0]

                def mk(i):
                    def em():
                        ti, dp = divmod(i, ND // 2)
                        w_t, x_t = terms[ti]
                        sl = slice(2 * dp, 2 * dp + 2)
                        nc.tensor.matmul(out_ps[:, :], w_t[:, sl, colsl],
                                         x_t[:, sl, :],
                                         start=(i == 0), stop=(i == n_tot - 1),
                                         perf_mode=DR, skip_group_check=True)
                    return em
                # order: iterate dp-major inside each term for locality
                for ti in range(3):
                    for dp in range(ND // 2):
                        ems.append(mk(ti * (ND // 2) + dp))
                return ems

            def proj_emitters(cb):
                """Returns (kt_emitters, vt_emitters) for block cb."""
                xh, xl = state.pop(("x", cb))
                kt = ps_a.tile([HD, BLK], F32, name=f"kt{cb}", tag="proj")
                vt = ps_a.tile([HD, BLK], F32, name=f"vt{cb}", tag="proj")
                kt_ems = dr_emitters(kt, wkh, wkl, xh, xl, slice(None))
                vt_ems = dr_emitters(vt, wvh, wvl, xh, xl, slice(None))
                state[("ktvt", cb)] = (kt, vt)
                return kt_ems, vt_ems

            def prep_block_v(cb):
                """bf16 copy + xbar transpose of V (block cb)."""
                vt = state.pop(("vt", cb))
                vt16 = scr.tile([HD, BLK], BF16, name=f"vt16{cb}", tag="vt16")
                nc.vector.tensor_copy(vt16[:, :], vt[:, :])
                vn = vnp.tile([128, 4, HD], BF16, name=f"vn{cb}", tag="vn")
                nc.sync.dma_start_transpose(vn[:, :, :], vt16[:, :])
                state[("vn", cb)] = vn

            def prep_block(cb, split_v=False):
                """Norm-stats + rope for K (block cb); V unless split_v."""
                kt, vt = state.pop(("ktvt", cb))
                state[("vt", cb)] = vt
                ck, sk = state.pop(("cs", cb))
                src16 = scr.tile([128, BLK], BF16, name=f"src{cb}", tag="src")
                nc.vector.tensor_copy(src16[:, :], kt[:, :])
                sq = scr.tile([128, BLK], BF16, name=f"sq{cb}", tag="sq")
                nc.gpsimd.tensor_mul(sq[:, :], src16[:, :], src16[:, :])
                ssq = ps_d.tile([128, 4], F32, name=f"ssq{cb}", tag="ssq")
                for j in range(4):
                    nc.tensor.matmul(ssq[:, j:j + 1],
                                     sq[:, j * 128:(j + 1) * 128],
                                     onescb[:, :], start=True, stop=True)
                u = scr.tile([128, 4], F32, name=f"u{cb}", tag="u")
                nc.scalar.activation(u[:, :], ssq[:, :], AF.Ln,
                                     bias=eps128[:, :], scale=SSQ_SC)
                a_k = scr.tile([128, 4], F32, name=f"ak{cb}", tag="ak", bufs=3)
                nc.scalar.activation(a_k[:, :], u[:, :], AF.Exp,
                                     bias=biask128[:, :], scale=-0.5)
                sh = scr.tile([128, BLK], BF16, name=f"sh{cb}", tag="sh")
                nc.vector.stream_shuffle(sh[:, :], src16[:, :], SWAP_MASK)
                t1 = scr.tile([128, BLK], BF16, name=f"t1{cb}", tag="t1")
                nc.gpsimd.tensor_mul(t1[:, :], src16[:, :], ck[:, :])
                t2 = scr.tile([128, BLK], BF16, name=f"t2{cb}", tag="t2")
                nc.vector.tensor_mul(t2[:, :], sh[:, :], sk[:, :])
                ktf = scr.tile([128, BLK], BF16, name=f"ktf{cb}", tag="ktf", bufs=3)
                nc.vector.tensor_add(ktf[:, :], t1[:, :], t2[:, :])
                state[("ktf", cb)] = ktf
                state[("ak", cb)] = a_k
                if not split_v:
                    prep_block_v(cb)

            def attn_block(cb, fillers, first=False, last=False):
                ktf = state.pop(("ktf", cb))
                a_k = state.pop(("ak", cb))
                vn = state.pop(("vn", cb))
                fi = [0]

                def fill(n):
                    for _ in range(n):
                        if fi[0] < len(fillers):
                            fillers[fi[0]]()
                            fi[0] += 1

                pend = []

                def normalize_head(h):
                    den = ps_d.tile([1, SQ], F32, name=f"den{h}", tag="ssq")
                    nc.tensor.matmul(den[:, :], onesc_r[:, :], pacc[h][:, :],
                                     start=True, stop=True, skip_group_check=True)
                    ud = scr.tile([1, SQ], F32, name=f"ud{h}", tag="u")
                    nc.scalar.activation(ud[:, :], den[:, :], AF.Ln,
                                         bias=zero1[:, :], scale=1.0)
                    rd = scr.tile([1, SQ], F32R, name=f"rd{h}", tag="rdt")
                    nc.scalar.activation(rd[:, :], ud[:, :], AF.Exp,
                                         bias=biasden1[:, :], scale=-1.0)
                    nfd = ps_c.tile([HD, SQ], F32, name=f"nfd{h}", tag="ot")
                    nc.tensor.matmul(nfd[:, :], onesr_r[:, :], rd[:, :],
                                     start=True, stop=True, skip_group_check=True)
                    t_n = scr.tile([HD, SQ], F32, name=f"tn{h}", tag="tn")
                    nc.vector.tensor_mul(t_n[:, :], otsb[h][:, :], nfd[:, :])
                    if h == 3:
                        nc.vector.tensor_copy(otn_hi[:, h, :], t_n[:, :])
                        nc.gpsimd.tensor_sub(otn_lo[:, h, :], t_n[:, :], otn_hi[:, h, :])
                    else:
                        nc.scalar.copy(otn_hi[:, h, :], t_n[:, :])
                        if h % 2 == 0:
                            nc.gpsimd.tensor_sub(otn_lo[:, h, :], t_n[:, :], otn_hi[:, h, :])
                        else:
                            nc.vector.tensor_sub(otn_lo[:, h, :], t_n[:, :], otn_hi[:, h, :])

                def flush_one():
                    h, j, p_t, ot_ps = pend.pop(0)
                    nc.tensor.matmul(ot_ps[:, :], vn[:, j, :], p_t[:, :],
                                     start=(j == 0), stop=(j == 3),
                                     skip_group_check=True)
                    if first and j == 0:
                        nc.gpsimd.tensor_copy(pacc[h][:, :], p_t[:, :])
                    elif last and j % 2 == 1:
                        nc.vector.tensor_add(pacc[h][:, :], pacc[h][:, :], p_t[:, :])
                    else:
                        nc.gpsimd.tensor_add(pacc[h][:, :], pacc[h][:, :], p_t[:, :])
                    if j == 3:
                        if first:
                            nc.vector.tensor_copy(otsb[h][:, :], ot_ps[:, :])
                        else:
                            nc.vector.tensor_add(otsb[h][:, :], otsb[h][:, :], ot_ps[:, :])
                    if last and j == 1 and h >= 1:
                        normalize_head(h - 1)

                for h in range(NH):
                    ot_ps = ps_c.tile([HD, SQ], F32, name=f"ot{cb}_{h}", tag="ot")
                    for j in range(4):
                        st = ps_b.tile([128, SQ], F32, name=f"st{cb}_{h}_{j}", tag="st")
                        nc.tensor.matmul(st[:, :], ktf[:, j * 128:(j + 1) * 128],
                                         qrope[h][:, :], start=True, stop=True,
                                         skip_group_check=True)
                        p_t = pex.tile([128, SQ], BF16, name=f"p{cb}_{h}_{j}", tag="p")
                        nc.scalar.activation(p_t[:, :], st[:, :], AF.Exp,
                                             bias=zero128[:, :],
                                             scale=a_k[:, j:j + 1])
                        pend.append((h, j, p_t, ot_ps))
                        fill(3)
                        if len(pend) >= pend_depth:
                            flush_one()
                while pend:
                    flush_one()
                fill(len(fillers))
                if last:
                    normalize_head(3)

            # ---------- phase 1: Q projection + norm + rope ----------
            # dp-major across heads: matmuls fire as DMA quarters land
            psqs = [(ps_b if h < 2 else ps_a).tile([HD, SQ], F32, name=f"psq{h}",
                                                   tag="st" if h < 2 else "proj")
                    for h in range(NH)]
            state[("x", 8)] = (xh8_t, xl8_t)
            load_cs(8)
            qterms = [(wqh_t, xh8_t), (wql_t, xh8_t), (wqh_t, xl8_t)]
            for ti, (w_t, x_t) in enumerate(qterms):
                for dp in range(ND // 2):
                    dsl = slice(2 * dp, 2 * dp + 2)
                    for h in range(NH):
                        hsl = slice(h * HD, (h + 1) * HD)
                        nc.tensor.matmul(
                            psqs[h][:, :], w_t[:, dsl, hsl], x_t[:, dsl, :],
                            start=(ti == 0 and dp == 0),
                            stop=(ti == 2 and dp == ND // 2 - 1),
                            perf_mode=DR, skip_group_check=True)
            for h in range(NH):
                psq = psqs[h]
                srcq = scr.tile([HD, SQ], F32R, name=f"srcq{h}", tag="src")
                nc.vector.tensor_copy(srcq[:, :], psq[:, :])
                sqq = scr.tile([HD, SQ], BF16, name=f"sqq{h}", tag="sq")
                nc.gpsimd.tensor_mul(sqq[:, :], srcq[:, :], srcq[:, :])
                ssqq = ps_c.tile([1, SQ], F32, name=f"ssqq{h}", tag="ot")
                nc.tensor.matmul(ssqq[:, :], onescb[:, :], sqq[:, :],
                                 start=True, stop=True)
                uq = scr.tile([1, SQ], F32, name=f"uq{h}", tag="u")
                nc.scalar.activation(uq[:, :], ssqq[:, :], AF.Ln,
                                     bias=eps1[:, :], scale=SSQ_SC)
                rsq = scr.tile([1, SQ], F32R, name=f"rsq{h}", tag="rsy")
                nc.scalar.activation(rsq[:, :], uq[:, :], AF.Exp,
                                     bias=biasq1[:, :], scale=-0.5)
                nfq = ps_c.tile([HD, SQ], F32, name=f"nfq{h}", tag="ot")
                nc.tensor.matmul(nfq[:, :], onesr_r[:, :], rsq[:, :],
                                 start=True, stop=True)
                xnq = scr.tile([HD, SQ], BF16, name=f"xnq{h}", tag="t1")
                nc.vector.tensor_mul(xnq[:, :], srcq[:, :], nfq[:, :])
                shq = scr.tile([HD, SQ], BF16, name=f"shq{h}", tag="sh")
                nc.vector.stream_shuffle(shq[:, :], xnq[:, :], SWAP_MASK)
                tq1 = scr.tile([HD, SQ], BF16, name=f"tq1{h}", tag="t2")
                nc.gpsimd.tensor_mul(tq1[:, :], xnq[:, :], cosq[:, :])
                tq2 = scr.tile([HD, SQ], BF16, name=f"tq2{h}", tag="vt16")
                nc.vector.tensor_mul(tq2[:, :], shq[:, :], sinq[:, :])
                nc.vector.tensor_add(qrope[h][:, :], tq1[:, :], tq2[:, :])

            # ---------- phase 2: kv block pipeline (block 8 first) ----------
            SEQ = [8] + list(range(NB - 1))
            load_block(0, split=True)
            load_block(1, split=True)
            k8, v8 = proj_emitters(8)
            for em in k8 + v8:
                em()
            prep_block(8)
            k0, v0 = proj_emitters(0)
            for em in k0 + v0:
                em()
            prep_block(0)
            for s in range(NB):
                bid = SEQ[s]
                if s + 3 < NB:
                    load_block(SEQ[s + 3])
                if s == NB - 2:
                    wo8h = bpool.tile([128, 4, NH, 512], FP8, name="wo8h")
                    nc.sync.dma_start(
                        wo8h[:, :, :, :],
                        wo_hi_d[:, :].rearrange("p (n h c) -> p n h c", n=4, h=NH))
                    wo8l = bpool.tile([128, 4, NH, 512], FP8, name="wo8l")
                    nc.sync.dma_start(
                        wo8l[:, :, :, :],
                        wo_lo_d[:, :].rearrange("p (n h c) -> p n h c", n=4, h=NH))
                if s + 2 < NB:
                    kt_ems, vt_ems = proj_emitters(SEQ[s + 2])
                    if s + 2 == NB - 1:
                        state["vt_last"] = vt_ems
                        fillers = kt_ems
                    else:
                        fillers = kt_ems + vt_ems
                elif s + 2 == NB:
                    fillers = state.pop("vt_last")
                else:
                    fillers = []
                attn_block(bid, fillers, first=(s == 0), last=(s == NB - 1))
                if s + 2 < NB:
                    prep_block(SEQ[s + 2], split_v=(s + 2 == NB - 1))
                elif s + 2 == NB:
                    prep_block_v(SEQ[NB - 1])

            # ---------- phase 3: o_proj ----------
            osbm = [scr.tile([128, D], BF16, name=f"osbm{m}", tag=f"osbm{m}", bufs=1)
                    for m in range(4)]
            for m in range(4):
                msl = slice(m * HD, (m + 1) * HD)
                for n in range(4):
                    po = ps_b.tile([128, 512], F32, name=f"po{m}_{n}", tag="st")
                    pterms = [(otn_hi, wo8h), (otn_lo, wo8h), (otn_hi, wo8l)]
                    for hp in range(2):
                        hsl2 = slice(2 * hp, 2 * hp + 2)
                        for ti, (o_t, w_t) in enumerate(pterms):
                            nc.tensor.matmul(po[:, :], o_t[:, hsl2, msl],
                                             w_t[:, n, hsl2, :],
                                             start=(hp == 0 and ti == 0),
                                             stop=(hp == 1 and ti == 2),
                                             perf_mode=DR, skip_group_check=True)
                    nsl = slice(n * 512, (n + 1) * 512)
                    RS = 1.0 / (WS * WS)
                    if (m * 4 + n) % 2 == 0:
                        nc.vector.tensor_scalar_mul(osbm[m][:, nsl], po[:, :], RS)
                    else:
                        nc.scalar.mul(osbm[m][:, nsl], po[:, :], RS)
                    oq = nc.sync if (m * 4 + n) % 2 == 0 else nc.scalar
                    oq.dma_start(out[m * 128:(m + 1) * 128, nsl], osbm[m][:, nsl])
    nc.finalize()
    return nc


def get_nc(**kw):
    key = ("nc2", tuple(sorted(kw.items())))
    if key not in _CACHE:
        _CACHE[key] = _build_nc(**kw)
    return _CACHE[key]


# ---------------- host-side data prep ----------------

INTL = np.empty(HD, np.int64)   # new slot -> original dim
INTL[0::2] = np.arange(64)
INTL[1::2] = np.arange(64) + 64


def _hi_lo(a32):
    hi = a32.astype(E4M3)
    lo = (a32 - hi.astype(np.float32)).astype(E4M3)
    return hi, lo


def _pack16(a):
    """[D, C] -> [128, ND*C] partition-major chunk layout."""
    Dd, C = a.shape
    return np.ascontiguousarray(
        a.reshape(ND, 128, C).transpose(1, 0, 2).reshape(128, ND * C))


def _rope_tables(pos, w):
    """pos [S], w [HD] -> cos/sin tables [HD, S] interleaved, w- and sign-folded."""
    inv = 1.0 / (THETA ** (np.arange(0, HD, 2, dtype=np.float32) / np.float32(HD)))
    ang = inv[:, None] * pos[None, :].astype(np.float32)   # [64, S]
    c, s = np.cos(ang), np.sin(ang)
    cosT = np.empty((HD, len(pos)), np.float32)
    sinT = np.empty((HD, len(pos)), np.float32)
    cosT[0::2] = w[:64, None] * c
    cosT[1::2] = w[64:, None] * c
    sinT[0::2] = -w[64:, None] * s   # partner dim's weight rides the sin term
    sinT[1::2] = w[:64, None] * s
    return cosT.astype(BF), sinT.astype(BF)


def _make_in_maps(inputs):
    draft = np.asarray(inputs["draft_hidden"], np.float32)
    ctx = np.asarray(inputs["context_hidden"], np.float32)
    Wq = np.asarray(inputs["Wq"], np.float32)
    Wk = np.asarray(inputs["Wk"], np.float32)
    Wv = np.asarray(inputs["Wv"], np.float32)
    Wo = np.asarray(inputs["Wo"], np.float32)
    qnw = np.asarray(inputs["q_norm_w"], np.float32).reshape(HD)
    knw = np.asarray(inputs["k_norm_w"], np.float32).reshape(HD)
    cpos = np.asarray(inputs["context_position_ids"])
    dpos = np.asarray(inputs["draft_position_ids"])

    in_maps = []
    xkv_c = {}
    xd_c = {}
    for c in range(8):
        b, g = c // 4, c % 4
        if b not in xkv_c:
            kvin = np.concatenate([ctx[b], draft[b]], axis=0)        # [TOT, D]
            xh, xl = _hi_lo(kvin.T)                                  # [D, TOT]
            xkv_c[b] = (_pack16(xh), _pack16(xl))
        xkv_hi, xkv_lo = xkv_c[b]

        # Wq for this core's 4 heads, head-dim interleaved, scaled, transposed
        Wqc = Wq.reshape(H, HD, D)[4 * g:4 * g + 4][:, INTL, :]      # [4,128,D]
        WqT = Wqc.reshape(4 * HD, D).T * WS                          # [D, 512]
        wqh, wql = _hi_lo(WqT)
        Wkc = Wk.reshape(KVH, HD, D)[g][INTL, :]                     # [128, D]
        WkT = Wkc.T * WS
        wkh, wkl = _hi_lo(WkT)
        WvT = Wv.reshape(KVH, HD, D)[g].T * WS                       # [D, 128]
        wvh, wvl = _hi_lo(WvT)

        fpos = np.concatenate([cpos[b], dpos[b]])
        cosk, sink = _rope_tables(fpos, knw)
        cosq, sinq = _rope_tables(dpos[b], qnw)

        # wo: [128p(hd), n, h, c] = Wo[n*512+c, (4g+h)*128+p]
        Woc = Wo[:, 4 * g * HD:(4 * g + 4) * HD]                     # [D, 512]
        woN = Woc.reshape(4, 512, NH, HD).transpose(3, 0, 2, 1)      # [128, 4n, h, 512c]
        woN = np.ascontiguousarray(woN.reshape(128, -1)) * WS
        wo_hi, wo_lo = _hi_lo(woN)

        in_maps.append({
            "xkv_hi": xkv_hi, "xkv_lo": xkv_lo,
            "wq_hi": _pack16(wqh), "wq_lo": _pack16(wql),
            "wk_hi": _pack16(wkh), "wk_lo": _pack16(wkl),
            "wv_hi": _pack16(wvh), "wv_lo": _pack16(wvl),
            "cosq": cosq, "sinq": sinq, "cosk": cosk, "sink": sink,
            "wo_hi": wo_hi, "wo_lo": wo_lo,
            "onesc": np.ones((128, 1), np.float32),
            "onesr": np.ones((1, HD), np.float32),
        })
    return in_maps


def kernel(**inputs):
    in_maps = _make_in_maps(inputs)
    nc = get_nc()
    res = bass_utils.run_bass_kernel_spmd(nc, in_maps, core_ids=list(range(8)))
    outs = [np.asarray(res.results[c]["out"], np.float32) for c in range(8)]
    full = np.stack([
        outs[0] + outs[1] + outs[2] + outs[3],
        outs[4] + outs[5] + outs[6] + outs[7],
    ]).astype(np.float32)
    return full
